# revision 41
# baseline (speedup 1.0000x reference)
"""Trainium2 Bass kernel for nn_Loss2_53996328845453 (segment_reduce).

Computes a multi-term image loss over B=16 samples of 512x512 images:
  total = 10*L_exp + 1*L_tv + 10*L_color + 50*L_sem

Strategy (pure data parallel, B sharded 2-per-core across 8 cores):
  - All inputs are pre-cast to fp16 on host (rel-err budget 2e-2 vs
    ~1e-4 incurred), halving HBM traffic to ~15.7MB/core and letting
    masks and I_enh DMA directly into the matmul operand tile.
  - Semantic/color terms: per-sample Gram matrix on the TensorEngine.
    X side (stationary, chunk-major fp16, 7 lanes): [R, R², 1]
    Y side (moving, map-major fp16, 19 lanes):      [M×8, M²×8, I×3]
    Q=16 chunks per matmul (lhsT 112 cols, rhs 304 cols); only the
    chunk-diagonal blocks of the [112,304] PSUM accumulation matter —
    dumped whole, diagonal extracted on host. ΣI and n=ΣM, ΣM² fall
    out of the X-ones row.
  - M² on DVE (fp16 2x); R reshuffle + R² on ACT; slabs interleaved
    across the two samples; masks on sync queue, R/I/L on gpsimd.
  - Exposure: 16-row group sums via PE pooling matmul; host finishes.
  - Vertical TV: PE bidiagonal-shift matmul + ACT abs-accumulate.
  - Final scalar assembly on host in float64 from tiny per-core outputs.
"""
import os
import sys

import numpy as np

try:
    import concourse.bacc as bacc  # noqa: F401
except ImportError:
    sys.path.insert(0, "/opt/trn_rl_repo")

from contextlib import ExitStack

import concourse.bacc as bacc
import concourse.tile as tile
from concourse import mybir
from concourse import bass_utils

# problem constants (hardcoded per spec)
B, NCORES = 16, 8
BLOC = B // NCORES            # 2 samples per core
H = W = 512
HW = H * W                    # 262144 px
K, C = 8, 3
P = 128                       # SBUF partitions / matmul contraction
FALL = HW // P                # 2048 chunks of 128 px per sample
XC, YC = 7, 19                # gram lanes per chunk (X stationary, Y moving)
Q = 16                        # chunks packed per matmul
BOUNDS = [0, 512, 1024, 1536, 1792, 2048]  # tapered slabs
NSLAB = len(BOUNDS) - 1
E_EXP = 0.6
PATCH = 16
L_EXP_W, L_TV_W, L_COLOR_W, L_SEM_W = 10.0, 1.0, 10.0, 50.0

f32 = mybir.dt.float32
f16 = mybir.dt.float16

_NC_CACHE = {}
LAST_RESULTS = None


def _build_nc():
    nc = bacc.Bacc("TRN2")
    L_d = nc.dram_tensor("L_loc", [BLOC, 1, H, W], f16, kind="ExternalInput")
    R_d = nc.dram_tensor("R_loc", [BLOC, C, H, W], f16, kind="ExternalInput")
    # masks and I_enh concatenated on host: [M0..M7, I0..I2]
    MI_d = nc.dram_tensor(
        "MI_loc", [BLOC, K + C, H, W], f16, kind="ExternalInput"
    )
    # constant bidiagonal shift matrix for vertical TV diffs on the PE
    S_d = nc.dram_tensor("shift_d", [P, P], f16, kind="ExternalInput")
    # constant 16-row pooling matrix for exposure partial sums on the PE
    Pool_d = nc.dram_tensor("pool_d", [P, 8], f16, kind="ExternalInput")
    # full [112,304] gram block per sample (host extracts chunk-diagonal)
    gram_o = nc.dram_tensor(
        "gram_o", [BLOC, XC * Q, YC * Q], f32, kind="ExternalOutput"
    )
    # L-path TV output: [:, 0:4] vertical TV band sums, [:, 4:8]
    # horizontal TV, [:, 8] band-boundary vertical sums (rows 0:3)
    lout_o = nc.dram_tensor("lout_o", [BLOC, P, 16], f32, kind="ExternalOutput")
    # exposure row-group partials: [8 groups, 4 bands * 512 w]
    eout_o = nc.dram_tensor("eout_o", [BLOC, 8, 4 * W], f32, kind="ExternalOutput")

    with ExitStack() as ctx:
        tc = ctx.enter_context(tile.TileContext(nc))
        rsp = ctx.enter_context(tc.tile_pool(name="rsp", bufs=4))
        xp = ctx.enter_context(tc.tile_pool(name="xp", bufs=4))
        yp = ctx.enter_context(tc.tile_pool(name="yp", bufs=4))
        lp = ctx.enter_context(tc.tile_pool(name="lp", bufs=1))
        sp = ctx.enter_context(tc.tile_pool(name="sp", bufs=1))
        op = ctx.enter_context(tc.tile_pool(name="op", bufs=2))
        cs = ctx.enter_context(tc.tile_pool(name="cs", bufs=1))
        pp = ctx.enter_context(tc.tile_pool(name="pp", bufs=2, space="PSUM"))
        vp = ctx.enter_context(tc.tile_pool(name="vp", bufs=2, space="PSUM"))

        # flat per-map HBM views: [128, nmaps, 2048]
        Rf, MIf = [], []
        for b in range(BLOC):
            Rf.append(
                R_d[b].rearrange("c h w -> c (h w)").rearrange(
                    "c (p f) -> p c f", p=P
                )
            )
            MIf.append(
                MI_d[b].rearrange("k h w -> k (h w)").rearrange(
                    "k (p f) -> p k f", p=P
                )
            )

        psum_g = [
            pp.tile([XC * Q, YC * Q], f32, tag=f"psum_g{b}", name=f"psum_g{b}")
            for b in range(BLOC)
        ]
        nmm_total = FALL // Q  # accumulation group length per sample

        Ssb = cs.tile([P, P], f16)
        Poolsb = cs.tile([P, 8], f16)

        def l_path(b):
            # ---- L path: exposure partials + TV partials (band-batched)
            Lb = L_d[b, 0]  # [512, 512]
            Lbands = Lb.rearrange("(r p) w -> p r w", p=P)      # [128,4,512]
            ot = op.tile([P, 16], f32, tag="ot")
            Lt = lp.tile([P, 4, W], f16, tag=f"Lt{b}")
            nc.gpsimd.dma_start(out=Lt, in_=Lbands)
            # band-boundary rows for vertical diffs (127,128),(255,256),(383,384)
            Ba = lp.tile([P, W], f16, tag=f"Ba{b}")
            Bb = lp.tile([P, W], f16, tag=f"Bb{b}")
            bnd = Lb.rearrange("(r p) w -> r p w", p=P)  # [4,128,512]
            nc.gpsimd.dma_start(out=Ba[0:3, :], in_=bnd[0:3, 127, :])
            nc.gpsimd.dma_start(out=Bb[0:3, :], in_=bnd[1:4, 0, :])
            # horizontal TV: wide sub on DVE (fp16 2x), abs-sums on ACT
            dh = sp.tile([P, 4, W], f16, tag="dh")
            trash = sp.tile([P, W], f16, tag="trash")
            nc.vector.tensor_sub(
                dh[:, :, 0 : W - 1], Lt[:, :, 1:W], Lt[:, :, 0 : W - 1]
            )
            nc.vector.tensor_reduce(
                ot[:, 4:8],
                dh[:, :, 0 : W - 1],
                axis=mybir.AxisListType.X,
                op=mybir.AluOpType.add,
                apply_absolute_value=True,
            )
            # vertical TV within bands: PE bidiagonal shift (fp16 weights
            # are exact ±1), row 127 of each product is zero.
            for r in range(4):
                psum_v = vp.tile([P, W], f32, tag="psum_v")
                nc.tensor.matmul(
                    psum_v, lhsT=Ssb, rhs=Lt[:, r, :], start=True, stop=True
                )
                nc.vector.tensor_reduce(
                    ot[:, r : r + 1],
                    psum_v,
                    axis=mybir.AxisListType.X,
                    op=mybir.AluOpType.add,
                    apply_absolute_value=True,
                )
                psum_e = vp.tile([8, W], f32, tag="psum_e")
                nc.tensor.matmul(
                    psum_e, lhsT=Poolsb, rhs=Lt[:, r, :], start=True, stop=True
                )
                eo = sp.tile([8, W], f32, tag="eo")
                nc.scalar.copy(eo, psum_e)
                nc.gpsimd.dma_start(
                    out=eout_o[b, :, r * W : (r + 1) * W], in_=eo
                )
            # vertical TV across band boundaries (3 rows)
            nc.vector.memset(ot[:, 8:16], 0.0)
            dv = sp.tile([P, W], f16, tag="dv")
            nc.vector.tensor_sub(dv[0:3, :], Bb[0:3, :], Ba[0:3, :])
            nc.vector.tensor_reduce(
                ot[0:3, 8:9],
                dv[0:3, :],
                axis=mybir.AxisListType.X,
                op=mybir.AluOpType.add,
                apply_absolute_value=True,
            )
            nc.gpsimd.dma_start(out=lout_o[b], in_=ot)

        for s in range(NSLAB):
            sl = slice(BOUNDS[s], BOUNDS[s + 1])
            Fs = BOUNDS[s + 1] - BOUNDS[s]
            for b in range(BLOC):
                # ---- Y tile lanes [M×8, I×3, M²×8]: masks+I arrive as
                # ONE sync-queue DMA straight into rows 0:11 (fp16)
                Y = yp.tile([P, YC, Fs], f16, tag="Y")
                nc.sync.dma_start(out=Y[:, 0 : K + C, :], in_=MIf[b][:, :, sl])
                Rs = rsp.tile([P, C, Fs], f16, tag="Rs")
                nc.gpsimd.dma_start(out=Rs, in_=Rf[b][:, :, sl])

                if s == 0:
                    # L-path after first slab loads are in flight; the
                    # constant matrices ride the scalar queue once
                    if b == 0:
                        nc.scalar.dma_start(out=Ssb, in_=S_d[:])
                        nc.scalar.dma_start(out=Poolsb, in_=Pool_d[:])
                    l_path(b)

                # ---- M² on DVE (fp16 2x) into rows 11:19
                nc.vector.tensor_mul(
                    Y[:, 11:19, :], Y[:, 0:8, :], Y[:, 0:8, :]
                )

                # ---- X side (stationary): [R, R², 1] chunk-major fp16
                # (weights AP must collapse to one free dim); reshuffle
                # and square on ACT, ones lane on DVE
                X = xp.tile([P, Fs, XC], f16, tag="X")
                nc.vector.memset(X[:, :, 6], 1.0)
                Rt = Rs.rearrange("p c f -> p f c")
                nc.scalar.copy(X[:, :, 0:3], Rt)
                nc.scalar.activation(
                    X[:, :, 3:6], Rt, mybir.ActivationFunctionType.Square
                )

                # ---- packed matmuls: Q chunks per instruction; weight
                # columns ordered (chunk, lane), moving (lane, chunk)
                for m in range(Fs // Q):
                    g = BOUNDS[s] // Q + m
                    j0 = m * Q
                    nc.tensor.matmul(
                        psum_g[b],
                        lhsT=X[:, j0 : j0 + Q, :],
                        rhs=Y[:, :, j0 : j0 + Q],
                        start=(g == 0),
                        stop=(g == nmm_total - 1),
                    )

                if s == NSLAB - 1:
                    # ---- evacuate gram: one PSUM copy + DMA per sample
                    gram_sb = op.tile([XC * Q, YC * Q], f32, tag="gram_sb")
                    nc.scalar.copy(gram_sb, psum_g[b])
                    nc.sync.dma_start(out=gram_o[b], in_=gram_sb)

    nc.finalize()
    return nc


def _get_nc():
    if "nc" not in _NC_CACHE:
        _NC_CACHE["nc"] = _build_nc()
    return _NC_CACHE["nc"]


def kernel(L, R, I_enh, semantic_masks):
    global LAST_RESULTS
    nc = _get_nc()

    # bidiagonal shift matrix: out[m] = L[m+1] - L[m] for m < 127
    S = np.zeros((P, P), dtype=np.float16)
    for m in range(P - 1):
        S[m + 1, m] = 1.0
        S[m, m] = -1.0
    # 16-row pooling matrix: col g sums partitions 16g..16g+15
    Pool = np.zeros((P, 8), dtype=np.float16)
    for p in range(P):
        Pool[p, p // 16] = 1.0

    L16 = np.asarray(L, dtype=np.float16)
    R16 = np.asarray(R, dtype=np.float16)
    MI16 = np.concatenate(
        [
            np.asarray(semantic_masks, dtype=np.float16),
            np.asarray(I_enh, dtype=np.float16),
        ],
        axis=1,
    )

    in_maps = []
    for i in range(NCORES):
        sl = slice(BLOC * i, BLOC * (i + 1))
        in_maps.append(
            {
                "L_loc": np.ascontiguousarray(L16[sl]),
                "R_loc": np.ascontiguousarray(R16[sl]),
                "MI_loc": np.ascontiguousarray(MI16[sl]),
                "shift_d": S,
                "pool_d": Pool,
            }
        )

    res = bass_utils.run_bass_kernel_spmd(
        nc, in_maps, core_ids=list(range(NCORES))
    )
    LAST_RESULTS = res

    # ---- host-side combine in float64
    exp_acc = 0.0
    tv_acc_v = 0.0
    tv_acc_h = 0.0
    col_acc = 0.0
    sem_acc = 0.0
    for core in range(NCORES):
        o = res.results[core]
        gram_d = o["gram_o"].astype(np.float64)  # [BLOC, 112, 304]
        lout = o["lout_o"].astype(np.float64)    # [BLOC, P, 16]
        eout = o["eout_o"].astype(np.float64)    # [BLOC, 8, 4*512]
        for b in range(BLOC):
            # diagonal extraction: value[q, xc, yc] = dump[q*XC+xc, yc*Q+q]
            g = np.einsum(
                "qxyq->xy", gram_d[b].reshape(Q, XC, YC, Q)
            )  # summed over q: [XC, YC]
            # X rows: 0:3 R, 3:6 R², 6 ones
            # Y cols: 0:8 M, 8:11 I, 11:19 M²
            sRM = g[0:3, 0:8]        # [c, k]
            sRM2 = g[0:3, 11:19]
            sR2M2 = g[3:6, 11:19]
            sumI = g[6, 8:11]
            nvec = g[6, 0:8] + 1e-6
            sM2 = g[6, 11:19]
            mean = sRM / nvec[None, :]
            var = (sR2M2 - 2.0 * mean * sRM2 + mean * mean * sM2[None, :]).sum(
                axis=0
            ) / nvec
            sem_acc += var.sum()

            mI = sumI / HW
            col_acc += (
                (mI[0] - mI[1]) ** 2 + (mI[0] - mI[2]) ** 2 + (mI[1] - mI[2]) ** 2
            )

            # exposure: eout[g, r*W + w] = 16-row sums; finish 16-wide
            # column sums on host -> patch (pr = 8r + g, pc = w // 16)
            patch = eout[b].reshape(8, 4, 32, PATCH).sum(axis=-1)
            Lp = patch / (PATCH * PATCH)
            exp_acc += ((Lp - E_EXP) ** 2).sum()

            tv_acc_v += lout[b, :, 0:4].sum() + lout[b, :, 8].sum()
            tv_acc_h += lout[b, :, 4:8].sum()

    L_exp = exp_acc / (B * 32 * 32)
    L_tv = tv_acc_v / (B * 1 * (H - 1) * W) + tv_acc_h / (B * 1 * H * (W - 1))
    L_color = col_acc / B
    L_sem = sem_acc / B
    total = (
        L_EXP_W * L_exp + L_TV_W * L_tv + L_COLOR_W * L_color + L_SEM_W * L_sem
    )
    return np.float32(total)


# revision 42
# speedup vs baseline: 1.0808x; 1.0808x over previous
"""Trainium2 Bass kernel for nn_Loss2_53996328845453 (segment_reduce).

Computes a multi-term image loss over B=16 samples of 512x512 images:
  total = 10*L_exp + 1*L_tv + 10*L_color + 50*L_sem

Strategy (pure data parallel, B sharded 2-per-core across 8 cores):
  - All inputs are pre-cast to fp16 on host (rel-err budget 2e-2 vs
    ~1e-4 incurred), halving HBM traffic to ~15.7MB/core and letting
    masks and I_enh DMA directly into the matmul operand tile.
  - Semantic/color terms: per-sample Gram matrix on the TensorEngine.
    X side (stationary, chunk-major fp16, 7 lanes): [R, R², 1]
    Y side (moving, map-major fp16, 19 lanes):      [M×8, I×3, M²×8]
    Q=16 chunks per matmul (lhsT 112 cols, rhs 304 cols); only the
    chunk-diagonal blocks of the [112,304] PSUM accumulation matter —
    dumped whole, diagonal extracted on host. ΣI and n=ΣM, ΣM² fall
    out of the X-ones row.
  - Masks+I are host-concatenated so each slab's Y loads with ONE
    sync-queue DMA straight into the operand tile; R on gpsimd.
  - M² on DVE (fp16 2x); R reshuffle + R² on ACT; slabs interleaved
    across the two samples with tapered sizes and 4-deep operand rings.
  - Exposure: 16-row group sums via PE pooling matmul; host finishes.
  - Vertical TV: PE bidiagonal-shift matmul + DVE abs-reduce.
  - Final scalar assembly on host in float64 from tiny per-core outputs.
"""
import os
import sys

import numpy as np

try:
    import concourse.bacc as bacc  # noqa: F401
except ImportError:
    sys.path.insert(0, "/opt/trn_rl_repo")

from contextlib import ExitStack

import concourse.bacc as bacc
import concourse.tile as tile
from concourse import mybir
from concourse import bass_utils

# problem constants (hardcoded per spec)
B, NCORES = 16, 8
BLOC = B // NCORES            # 2 samples per core
H = W = 512
HW = H * W                    # 262144 px
K, C = 8, 3
P = 128                       # SBUF partitions / matmul contraction
FALL = HW // P                # 2048 chunks of 128 px per sample
XC, YC = 7, 19                # gram lanes per chunk (X stationary, Y moving)
Q = 16                        # chunks packed per matmul
BOUNDS = [0, 512, 1024, 1536, 1792, 2048]  # tapered slabs
NSLAB = len(BOUNDS) - 1
E_EXP = 0.6
PATCH = 16
L_EXP_W, L_TV_W, L_COLOR_W, L_SEM_W = 10.0, 1.0, 10.0, 50.0

f32 = mybir.dt.float32
f16 = mybir.dt.float16

_NC_CACHE = {}
LAST_RESULTS = None


def _build_nc():
    nc = bacc.Bacc("TRN2")
    L_d = nc.dram_tensor("L_loc", [BLOC, 1, H, W], f16, kind="ExternalInput")
    R_d = nc.dram_tensor("R_loc", [BLOC, C, H, W], f16, kind="ExternalInput")
    # masks and I_enh concatenated on host: [M0..M7, I0..I2]
    MI_d = nc.dram_tensor(
        "MI_loc", [BLOC, K + C, H, W], f16, kind="ExternalInput"
    )
    # constant bidiagonal shift matrix for vertical TV diffs on the PE
    S_d = nc.dram_tensor("shift_d", [P, P], f16, kind="ExternalInput")
    # constant 16-row pooling matrix for exposure partial sums on the PE
    Pool_d = nc.dram_tensor("pool_d", [P, 8], f16, kind="ExternalInput")
    # full [112,304] gram block per sample (host extracts chunk-diagonal)
    gram_o = nc.dram_tensor(
        "gram_o", [BLOC, XC * Q, YC * Q], f32, kind="ExternalOutput"
    )
    # L-path TV output: [:, 0:4] vertical TV band sums, [:, 4:8]
    # horizontal TV, [:, 8] band-boundary vertical sums (rows 0:3)
    lout_o = nc.dram_tensor("lout_o", [BLOC, P, 16], f32, kind="ExternalOutput")
    # exposure row-group partials: [8 groups, 4 bands * 512 w]
    eout_o = nc.dram_tensor("eout_o", [BLOC, 8, 4 * W], f32, kind="ExternalOutput")

    with ExitStack() as ctx:
        tc = ctx.enter_context(tile.TileContext(nc))
        rsp = ctx.enter_context(tc.tile_pool(name="rsp", bufs=4))
        xp = ctx.enter_context(tc.tile_pool(name="xp", bufs=4))
        yp = ctx.enter_context(tc.tile_pool(name="yp", bufs=4))
        lp = ctx.enter_context(tc.tile_pool(name="lp", bufs=1))
        sp = ctx.enter_context(tc.tile_pool(name="sp", bufs=1))
        op = ctx.enter_context(tc.tile_pool(name="op", bufs=2))
        cs = ctx.enter_context(tc.tile_pool(name="cs", bufs=1))
        pp = ctx.enter_context(tc.tile_pool(name="pp", bufs=2, space="PSUM"))
        vp = ctx.enter_context(tc.tile_pool(name="vp", bufs=2, space="PSUM"))

        # flat per-map HBM views: [128, nmaps, 2048]
        Rf, MIf = [], []
        for b in range(BLOC):
            Rf.append(
                R_d[b].rearrange("c h w -> c (h w)").rearrange(
                    "c (p f) -> p c f", p=P
                )
            )
            MIf.append(
                MI_d[b].rearrange("k h w -> k (h w)").rearrange(
                    "k (p f) -> p k f", p=P
                )
            )

        psum_g = [
            pp.tile([XC * Q, YC * Q], f32, tag=f"psum_g{b}", name=f"psum_g{b}")
            for b in range(BLOC)
        ]
        nmm_total = FALL // Q  # accumulation group length per sample

        Ssb = cs.tile([P, P], f16)
        Poolsb = cs.tile([P, 8], f16)

        def l_path(b):
            # ---- L path: exposure partials + TV partials (band-batched)
            Lb = L_d[b, 0]  # [512, 512]
            Lbands = Lb.rearrange("(r p) w -> p r w", p=P)      # [128,4,512]
            ot = op.tile([P, 16], f32, tag="ot")
            Lt = lp.tile([P, 4, W], f16, tag=f"Lt{b}")
            nc.gpsimd.dma_start(out=Lt, in_=Lbands)
            # band-boundary rows for vertical diffs (127,128),(255,256),(383,384)
            Ba = lp.tile([P, W], f16, tag=f"Ba{b}")
            Bb = lp.tile([P, W], f16, tag=f"Bb{b}")
            bnd = Lb.rearrange("(r p) w -> r p w", p=P)  # [4,128,512]
            nc.gpsimd.dma_start(out=Ba[0:3, :], in_=bnd[0:3, 127, :])
            nc.gpsimd.dma_start(out=Bb[0:3, :], in_=bnd[1:4, 0, :])
            # horizontal TV: wide sub on DVE (fp16 2x), abs-sums on ACT
            dh = sp.tile([P, 4, W], f16, tag="dh")
            trash = sp.tile([P, W], f16, tag="trash")
            nc.vector.tensor_sub(
                dh[:, :, 0 : W - 1], Lt[:, :, 1:W], Lt[:, :, 0 : W - 1]
            )
            nc.vector.tensor_reduce(
                ot[:, 4:8],
                dh[:, :, 0 : W - 1],
                axis=mybir.AxisListType.X,
                op=mybir.AluOpType.add,
                apply_absolute_value=True,
            )
            # vertical TV within bands: PE bidiagonal shift (fp16 weights
            # are exact ±1), row 127 of each product is zero.
            for r in range(4):
                psum_v = vp.tile([P, W], f32, tag="psum_v")
                nc.tensor.matmul(
                    psum_v, lhsT=Ssb, rhs=Lt[:, r, :], start=True, stop=True
                )
                nc.vector.tensor_reduce(
                    ot[:, r : r + 1],
                    psum_v,
                    axis=mybir.AxisListType.X,
                    op=mybir.AluOpType.add,
                    apply_absolute_value=True,
                )
                psum_e = vp.tile([8, W], f32, tag="psum_e")
                nc.tensor.matmul(
                    psum_e, lhsT=Poolsb, rhs=Lt[:, r, :], start=True, stop=True
                )
                eo = sp.tile([8, W], f32, tag="eo")
                nc.scalar.copy(eo, psum_e)
                nc.gpsimd.dma_start(
                    out=eout_o[b, :, r * W : (r + 1) * W], in_=eo
                )
            # vertical TV across band boundaries (3 rows)
            nc.vector.memset(ot[:, 8:16], 0.0)
            dv = sp.tile([P, W], f16, tag="dv")
            nc.vector.tensor_sub(dv[0:3, :], Bb[0:3, :], Ba[0:3, :])
            nc.vector.tensor_reduce(
                ot[0:3, 8:9],
                dv[0:3, :],
                axis=mybir.AxisListType.X,
                op=mybir.AluOpType.add,
                apply_absolute_value=True,
            )
            nc.gpsimd.dma_start(out=lout_o[b], in_=ot)

        for s in range(NSLAB):
            sl = slice(BOUNDS[s], BOUNDS[s + 1])
            Fs = BOUNDS[s + 1] - BOUNDS[s]
            for b in range(BLOC):
                # ---- Y tile lanes [M×8, I×3, M²×8]: masks+I arrive as
                # ONE sync-queue DMA straight into rows 0:11 (fp16)
                Y = yp.tile([P, YC, Fs], f16, tag="Y")
                nc.sync.dma_start(out=Y[:, 0 : K + C, :], in_=MIf[b][:, :, sl])
                Rs = rsp.tile([P, C, Fs], f16, tag="Rs")
                nc.gpsimd.dma_start(out=Rs, in_=Rf[b][:, :, sl])

                if s == 0:
                    # L-path after first slab loads are in flight; the
                    # constant matrices ride the scalar queue once
                    if b == 0:
                        nc.scalar.dma_start(out=Ssb, in_=S_d[:])
                        nc.scalar.dma_start(out=Poolsb, in_=Pool_d[:])
                    l_path(b)

                # ---- M² on DVE (fp16 2x) into rows 11:19
                nc.vector.tensor_mul(
                    Y[:, 11:19, :], Y[:, 0:8, :], Y[:, 0:8, :]
                )

                # ---- X side (stationary): [R, R², 1] chunk-major fp16
                # (weights AP must collapse to one free dim); reshuffle
                # and square on ACT, ones lane on DVE
                X = xp.tile([P, Fs, XC], f16, tag="X")
                nc.vector.memset(X[:, :, 6], 1.0)
                Rt = Rs.rearrange("p c f -> p f c")
                nc.scalar.copy(X[:, :, 0:3], Rt)
                nc.scalar.activation(
                    X[:, :, 3:6], Rt, mybir.ActivationFunctionType.Square
                )

                # ---- packed matmuls: Q chunks per instruction; weight
                # columns ordered (chunk, lane), moving (lane, chunk)
                for m in range(Fs // Q):
                    g = BOUNDS[s] // Q + m
                    j0 = m * Q
                    nc.tensor.matmul(
                        psum_g[b],
                        lhsT=X[:, j0 : j0 + Q, :],
                        rhs=Y[:, :, j0 : j0 + Q],
                        start=(g == 0),
                        stop=(g == nmm_total - 1),
                    )

                if s == NSLAB - 1:
                    # ---- evacuate gram: one PSUM copy + DMA per sample
                    gram_sb = op.tile([XC * Q, YC * Q], f32, tag="gram_sb")
                    nc.scalar.copy(gram_sb, psum_g[b])
                    nc.sync.dma_start(out=gram_o[b], in_=gram_sb)

    nc.finalize()
    return nc


def _get_nc():
    if "nc" not in _NC_CACHE:
        _NC_CACHE["nc"] = _build_nc()
    return _NC_CACHE["nc"]


def kernel(L, R, I_enh, semantic_masks):
    global LAST_RESULTS
    nc = _get_nc()

    # bidiagonal shift matrix: out[m] = L[m+1] - L[m] for m < 127
    S = np.zeros((P, P), dtype=np.float16)
    for m in range(P - 1):
        S[m + 1, m] = 1.0
        S[m, m] = -1.0
    # 16-row pooling matrix: col g sums partitions 16g..16g+15
    Pool = np.zeros((P, 8), dtype=np.float16)
    for p in range(P):
        Pool[p, p // 16] = 1.0

    L16 = np.asarray(L, dtype=np.float16)
    R16 = np.asarray(R, dtype=np.float16)
    MI16 = np.concatenate(
        [
            np.asarray(semantic_masks, dtype=np.float16),
            np.asarray(I_enh, dtype=np.float16),
        ],
        axis=1,
    )

    in_maps = []
    for i in range(NCORES):
        sl = slice(BLOC * i, BLOC * (i + 1))
        in_maps.append(
            {
                "L_loc": np.ascontiguousarray(L16[sl]),
                "R_loc": np.ascontiguousarray(R16[sl]),
                "MI_loc": np.ascontiguousarray(MI16[sl]),
                "shift_d": S,
                "pool_d": Pool,
            }
        )

    res = bass_utils.run_bass_kernel_spmd(
        nc, in_maps, core_ids=list(range(NCORES))
    )
    LAST_RESULTS = res

    # ---- host-side combine in float64
    exp_acc = 0.0
    tv_acc_v = 0.0
    tv_acc_h = 0.0
    col_acc = 0.0
    sem_acc = 0.0
    for core in range(NCORES):
        o = res.results[core]
        gram_d = o["gram_o"].astype(np.float64)  # [BLOC, 112, 304]
        lout = o["lout_o"].astype(np.float64)    # [BLOC, P, 16]
        eout = o["eout_o"].astype(np.float64)    # [BLOC, 8, 4*512]
        for b in range(BLOC):
            # diagonal extraction: value[q, xc, yc] = dump[q*XC+xc, yc*Q+q]
            g = np.einsum(
                "qxyq->xy", gram_d[b].reshape(Q, XC, YC, Q)
            )  # summed over q: [XC, YC]
            # X rows: 0:3 R, 3:6 R², 6 ones
            # Y cols: 0:8 M, 8:11 I, 11:19 M²
            sRM = g[0:3, 0:8]        # [c, k]
            sRM2 = g[0:3, 11:19]
            sR2M2 = g[3:6, 11:19]
            sumI = g[6, 8:11]
            nvec = g[6, 0:8] + 1e-6
            sM2 = g[6, 11:19]
            mean = sRM / nvec[None, :]
            var = (sR2M2 - 2.0 * mean * sRM2 + mean * mean * sM2[None, :]).sum(
                axis=0
            ) / nvec
            sem_acc += var.sum()

            mI = sumI / HW
            col_acc += (
                (mI[0] - mI[1]) ** 2 + (mI[0] - mI[2]) ** 2 + (mI[1] - mI[2]) ** 2
            )

            # exposure: eout[g, r*W + w] = 16-row sums; finish 16-wide
            # column sums on host -> patch (pr = 8r + g, pc = w // 16)
            patch = eout[b].reshape(8, 4, 32, PATCH).sum(axis=-1)
            Lp = patch / (PATCH * PATCH)
            exp_acc += ((Lp - E_EXP) ** 2).sum()

            tv_acc_v += lout[b, :, 0:4].sum() + lout[b, :, 8].sum()
            tv_acc_h += lout[b, :, 4:8].sum()

    L_exp = exp_acc / (B * 32 * 32)
    L_tv = tv_acc_v / (B * 1 * (H - 1) * W) + tv_acc_h / (B * 1 * H * (W - 1))
    L_color = col_acc / B
    L_sem = sem_acc / B
    total = (
        L_EXP_W * L_exp + L_TV_W * L_tv + L_COLOR_W * L_color + L_SEM_W * L_sem
    )
    return np.float32(total)


# revision 43
# speedup vs baseline: 1.1865x; 1.0978x over previous
"""Trainium2 Bass kernel for nn_Loss2_53996328845453 (segment_reduce).

Computes a multi-term image loss over B=16 samples of 512x512 images:
  total = 10*L_exp + 1*L_tv + 10*L_color + 50*L_sem

Strategy (pure data parallel, B sharded 2-per-core across 8 cores):
  - All inputs are pre-cast to fp16 on host (rel-err budget 2e-2 vs
    ~1e-4 incurred), halving HBM traffic to ~15.7MB/core and letting
    masks and I_enh DMA directly into the matmul operand tile.
  - Semantic/color terms: per-sample Gram matrix on the TensorEngine.
    X side (stationary, chunk-major fp16, 7 lanes): [R, R², 1]
    Y side (moving, map-major fp16, 19 lanes):      [M×8, I×3, M²×8]
    Q=16 chunks per matmul (lhsT 112 cols, rhs 304 cols); only the
    chunk-diagonal blocks of the [112,304] PSUM accumulation matter —
    dumped whole, diagonal extracted on host. ΣI and n=ΣM, ΣM² fall
    out of the X-ones row.
  - Masks+I are host-concatenated so each slab's Y loads with ONE
    sync-queue DMA straight into the operand tile; R on gpsimd.
  - M² on DVE (fp16 2x); R reshuffle + R² on ACT; slabs interleaved
    across the two samples with tapered sizes and 4-deep operand rings.
  - Exposure: 16-row group sums via PE pooling matmul; host finishes.
  - Vertical TV: PE bidiagonal-shift matmul + DVE abs-reduce.
  - Final scalar assembly on host in float64 from tiny per-core outputs.
"""
import os
import sys

import numpy as np

try:
    import concourse.bacc as bacc  # noqa: F401
except ImportError:
    sys.path.insert(0, "/opt/trn_rl_repo")

from contextlib import ExitStack

import concourse.bacc as bacc
import concourse.tile as tile
from concourse import mybir
from concourse import bass_utils

# problem constants (hardcoded per spec)
B, NCORES = 16, 8
BLOC = B // NCORES            # 2 samples per core
H = W = 512
HW = H * W                    # 262144 px
K, C = 8, 3
P = 128                       # SBUF partitions / matmul contraction
FALL = HW // P                # 2048 chunks of 128 px per sample
XC, YC = 7, 19                # gram lanes per chunk (X stationary, Y moving)
Q = 16                        # chunks packed per matmul
BOUNDS = [0, 512, 1024, 1536, 1792, 2048]  # tapered slabs
NSLAB = len(BOUNDS) - 1
E_EXP = 0.6
PATCH = 16
L_EXP_W, L_TV_W, L_COLOR_W, L_SEM_W = 10.0, 1.0, 10.0, 50.0

f32 = mybir.dt.float32
f16 = mybir.dt.float16

_NC_CACHE = {}
LAST_RESULTS = None


def _build_nc():
    nc = bacc.Bacc("TRN2")
    L_d = nc.dram_tensor("L_loc", [BLOC, 1, H, W], f16, kind="ExternalInput")
    R_d = nc.dram_tensor("R_loc", [BLOC, C, H, W], f16, kind="ExternalInput")
    # masks and I_enh concatenated on host: [M0..M7, I0..I2]
    MI_d = nc.dram_tensor(
        "MI_loc", [BLOC, K + C, H, W], f16, kind="ExternalInput"
    )
    # constant bidiagonal shift matrix for vertical TV diffs on the PE
    S_d = nc.dram_tensor("shift_d", [P, P], f16, kind="ExternalInput")
    # constant 16-row pooling matrix for exposure partial sums on the PE
    Pool_d = nc.dram_tensor("pool_d", [P, 8], f16, kind="ExternalInput")
    # full [112,304] gram block per sample (host extracts chunk-diagonal)
    gram_o = nc.dram_tensor(
        "gram_o", [BLOC, XC * Q, YC * Q], f32, kind="ExternalOutput"
    )
    # L-path TV output: [:, 0:4] vertical TV band sums, [:, 4:8]
    # horizontal TV, [:, 8] band-boundary vertical sums (rows 0:3)
    lout_o = nc.dram_tensor("lout_o", [BLOC, P, 16], f32, kind="ExternalOutput")
    # exposure row-group partials: [8 groups, 4 bands * 512 w]
    eout_o = nc.dram_tensor("eout_o", [BLOC, 8, 4 * W], f32, kind="ExternalOutput")

    with ExitStack() as ctx:
        tc = ctx.enter_context(tile.TileContext(nc))
        rsp = ctx.enter_context(tc.tile_pool(name="rsp", bufs=4))
        xp = ctx.enter_context(tc.tile_pool(name="xp", bufs=4))
        yp = ctx.enter_context(tc.tile_pool(name="yp", bufs=4))
        lp = ctx.enter_context(tc.tile_pool(name="lp", bufs=1))
        sp = ctx.enter_context(tc.tile_pool(name="sp", bufs=1))
        op = ctx.enter_context(tc.tile_pool(name="op", bufs=2))
        cs = ctx.enter_context(tc.tile_pool(name="cs", bufs=1))
        pp = ctx.enter_context(tc.tile_pool(name="pp", bufs=2, space="PSUM"))
        vp = ctx.enter_context(tc.tile_pool(name="vp", bufs=2, space="PSUM"))

        # flat per-map HBM views: [128, nmaps, 2048]
        Rf, MIf = [], []
        for b in range(BLOC):
            Rf.append(
                R_d[b].rearrange("c h w -> c (h w)").rearrange(
                    "c (p f) -> p c f", p=P
                )
            )
            MIf.append(
                MI_d[b].rearrange("k h w -> k (h w)").rearrange(
                    "k (p f) -> p k f", p=P
                )
            )

        psum_g = [
            pp.tile([XC * Q, YC * Q], f32, tag=f"psum_g{b}", name=f"psum_g{b}")
            for b in range(BLOC)
        ]
        nmm_total = FALL // Q  # accumulation group length per sample

        Ssb = cs.tile([P, P], f16)
        Poolsb = cs.tile([P, 8], f16)

        def l_path(b):
            # ---- L path: exposure partials + TV partials (band-batched)
            Lb = L_d[b, 0]  # [512, 512]
            Lbands = Lb.rearrange("(r p) w -> p r w", p=P)      # [128,4,512]
            ot = op.tile([P, 16], f32, tag="ot")
            Lt = lp.tile([P, 4, W], f16, tag=f"Lt{b}")
            nc.gpsimd.dma_start(out=Lt, in_=Lbands)
            # band-boundary rows for vertical diffs (127,128),(255,256),(383,384)
            Ba = lp.tile([P, W], f16, tag=f"Ba{b}")
            Bb = lp.tile([P, W], f16, tag=f"Bb{b}")
            bnd = Lb.rearrange("(r p) w -> r p w", p=P)  # [4,128,512]
            nc.gpsimd.dma_start(out=Ba[0:3, :], in_=bnd[0:3, 127, :])
            nc.gpsimd.dma_start(out=Bb[0:3, :], in_=bnd[1:4, 0, :])
            # horizontal TV: wide sub on DVE (fp16 2x), abs-sums on ACT
            dh = sp.tile([P, 4, W], f16, tag="dh")
            trash = sp.tile([P, W], f16, tag="trash")
            nc.vector.tensor_sub(
                dh[:, :, 0 : W - 1], Lt[:, :, 1:W], Lt[:, :, 0 : W - 1]
            )
            nc.vector.tensor_reduce(
                ot[:, 4:8],
                dh[:, :, 0 : W - 1],
                axis=mybir.AxisListType.X,
                op=mybir.AluOpType.add,
                apply_absolute_value=True,
            )
            # vertical TV within bands: PE bidiagonal shift (fp16 weights
            # are exact ±1), row 127 of each product is zero.
            for r in range(4):
                psum_v = vp.tile([P, W], f32, tag="psum_v")
                nc.tensor.matmul(
                    psum_v, lhsT=Ssb, rhs=Lt[:, r, :], start=True, stop=True
                )
                nc.vector.tensor_reduce(
                    ot[:, r : r + 1],
                    psum_v,
                    axis=mybir.AxisListType.X,
                    op=mybir.AluOpType.add,
                    apply_absolute_value=True,
                )
                psum_e = vp.tile([8, W], f32, tag="psum_e")
                nc.tensor.matmul(
                    psum_e, lhsT=Poolsb, rhs=Lt[:, r, :], start=True, stop=True
                )
                eo = sp.tile([8, W], f32, tag="eo")
                nc.scalar.copy(eo, psum_e)
                nc.gpsimd.dma_start(
                    out=eout_o[b, :, r * W : (r + 1) * W], in_=eo
                )
            # vertical TV across band boundaries (3 rows)
            nc.vector.memset(ot[:, 8:16], 0.0)
            dv = sp.tile([P, W], f16, tag="dv")
            nc.vector.tensor_sub(dv[0:3, :], Bb[0:3, :], Ba[0:3, :])
            nc.vector.tensor_reduce(
                ot[0:3, 8:9],
                dv[0:3, :],
                axis=mybir.AxisListType.X,
                op=mybir.AluOpType.add,
                apply_absolute_value=True,
            )
            nc.gpsimd.dma_start(out=lout_o[b], in_=ot)

        for s in range(NSLAB):
            sl = slice(BOUNDS[s], BOUNDS[s + 1])
            Fs = BOUNDS[s + 1] - BOUNDS[s]
            for b in range(BLOC):
                # ---- Y tile lanes [M×8, I×3, M²×8]: masks+I arrive as
                # ONE sync-queue DMA straight into rows 0:11 (fp16)
                Y = yp.tile([P, YC, Fs], f16, tag="Y")
                nc.sync.dma_start(out=Y[:, 0 : K + C, :], in_=MIf[b][:, :, sl])
                Rs = rsp.tile([P, C, Fs], f16, tag="Rs")
                nc.gpsimd.dma_start(out=Rs, in_=Rf[b][:, :, sl])

                # L-path: b=0 at startup, b=1 mid-stream where the
                # vector/scalar engines have slack
                if s == 0 and b == 0:
                    nc.scalar.dma_start(out=Ssb, in_=S_d[:])
                    nc.scalar.dma_start(out=Poolsb, in_=Pool_d[:])
                    l_path(0)
                elif s == 2 and b == 0:
                    l_path(1)

                # ---- M² on DVE (fp16 2x) into rows 11:19
                nc.vector.tensor_mul(
                    Y[:, 11:19, :], Y[:, 0:8, :], Y[:, 0:8, :]
                )

                # ---- X side (stationary): [R, R², 1] chunk-major fp16
                # (weights AP must collapse to one free dim); reshuffle
                # and square on ACT, ones lane on DVE
                X = xp.tile([P, Fs, XC], f16, tag="X")
                nc.vector.memset(X[:, :, 6], 1.0)
                Rt = Rs.rearrange("p c f -> p f c")
                nc.scalar.copy(X[:, :, 0:3], Rt)
                nc.scalar.activation(
                    X[:, :, 3:6], Rt, mybir.ActivationFunctionType.Square
                )

                # ---- packed matmuls: Q chunks per instruction; weight
                # columns ordered (chunk, lane), moving (lane, chunk)
                for m in range(Fs // Q):
                    g = BOUNDS[s] // Q + m
                    j0 = m * Q
                    nc.tensor.matmul(
                        psum_g[b],
                        lhsT=X[:, j0 : j0 + Q, :],
                        rhs=Y[:, :, j0 : j0 + Q],
                        start=(g == 0),
                        stop=(g == nmm_total - 1),
                    )

                if s == NSLAB - 1:
                    # ---- evacuate gram: one PSUM copy + DMA per sample
                    gram_sb = op.tile([XC * Q, YC * Q], f32, tag="gram_sb")
                    nc.scalar.copy(gram_sb, psum_g[b])
                    nc.sync.dma_start(out=gram_o[b], in_=gram_sb)

    nc.finalize()
    return nc


def _get_nc():
    if "nc" not in _NC_CACHE:
        _NC_CACHE["nc"] = _build_nc()
    return _NC_CACHE["nc"]


def kernel(L, R, I_enh, semantic_masks):
    global LAST_RESULTS
    nc = _get_nc()

    # bidiagonal shift matrix: out[m] = L[m+1] - L[m] for m < 127
    S = np.zeros((P, P), dtype=np.float16)
    for m in range(P - 1):
        S[m + 1, m] = 1.0
        S[m, m] = -1.0
    # 16-row pooling matrix: col g sums partitions 16g..16g+15
    Pool = np.zeros((P, 8), dtype=np.float16)
    for p in range(P):
        Pool[p, p // 16] = 1.0

    L16 = np.asarray(L, dtype=np.float16)
    R16 = np.asarray(R, dtype=np.float16)
    MI16 = np.concatenate(
        [
            np.asarray(semantic_masks, dtype=np.float16),
            np.asarray(I_enh, dtype=np.float16),
        ],
        axis=1,
    )

    in_maps = []
    for i in range(NCORES):
        sl = slice(BLOC * i, BLOC * (i + 1))
        in_maps.append(
            {
                "L_loc": np.ascontiguousarray(L16[sl]),
                "R_loc": np.ascontiguousarray(R16[sl]),
                "MI_loc": np.ascontiguousarray(MI16[sl]),
                "shift_d": S,
                "pool_d": Pool,
            }
        )

    res = bass_utils.run_bass_kernel_spmd(
        nc, in_maps, core_ids=list(range(NCORES))
    )
    LAST_RESULTS = res

    # ---- host-side combine in float64
    exp_acc = 0.0
    tv_acc_v = 0.0
    tv_acc_h = 0.0
    col_acc = 0.0
    sem_acc = 0.0
    for core in range(NCORES):
        o = res.results[core]
        gram_d = o["gram_o"].astype(np.float64)  # [BLOC, 112, 304]
        lout = o["lout_o"].astype(np.float64)    # [BLOC, P, 16]
        eout = o["eout_o"].astype(np.float64)    # [BLOC, 8, 4*512]
        for b in range(BLOC):
            # diagonal extraction: value[q, xc, yc] = dump[q*XC+xc, yc*Q+q]
            g = np.einsum(
                "qxyq->xy", gram_d[b].reshape(Q, XC, YC, Q)
            )  # summed over q: [XC, YC]
            # X rows: 0:3 R, 3:6 R², 6 ones
            # Y cols: 0:8 M, 8:11 I, 11:19 M²
            sRM = g[0:3, 0:8]        # [c, k]
            sRM2 = g[0:3, 11:19]
            sR2M2 = g[3:6, 11:19]
            sumI = g[6, 8:11]
            nvec = g[6, 0:8] + 1e-6
            sM2 = g[6, 11:19]
            mean = sRM / nvec[None, :]
            var = (sR2M2 - 2.0 * mean * sRM2 + mean * mean * sM2[None, :]).sum(
                axis=0
            ) / nvec
            sem_acc += var.sum()

            mI = sumI / HW
            col_acc += (
                (mI[0] - mI[1]) ** 2 + (mI[0] - mI[2]) ** 2 + (mI[1] - mI[2]) ** 2
            )

            # exposure: eout[g, r*W + w] = 16-row sums; finish 16-wide
            # column sums on host -> patch (pr = 8r + g, pc = w // 16)
            patch = eout[b].reshape(8, 4, 32, PATCH).sum(axis=-1)
            Lp = patch / (PATCH * PATCH)
            exp_acc += ((Lp - E_EXP) ** 2).sum()

            tv_acc_v += lout[b, :, 0:4].sum() + lout[b, :, 8].sum()
            tv_acc_h += lout[b, :, 4:8].sum()

    L_exp = exp_acc / (B * 32 * 32)
    L_tv = tv_acc_v / (B * 1 * (H - 1) * W) + tv_acc_h / (B * 1 * H * (W - 1))
    L_color = col_acc / B
    L_sem = sem_acc / B
    total = (
        L_EXP_W * L_exp + L_TV_W * L_tv + L_COLOR_W * L_color + L_SEM_W * L_sem
    )
    return np.float32(total)
